# revision 14
# baseline (speedup 1.0000x reference)
"""Trainium2 Bass kernel for MultiHeadAttention with DTW-cost bias.

Computes, per batch:
  q = Q @ Wq.T + bq ; k = K @ Wk.T + bk ; v = V @ Wv.T + bv   (split into 8 heads of 64)
  scores = q k^T / 8 + bias,  bias = einsum('cqk,hc->hqk', dtw, Wd) + bd
  attn = softmax(scores, axis=-1)
  out = (attn @ v heads concat) @ Wo.T + bo
Returns (out [16,512,512], attn [16,8,512,512]).

Sharding: data-parallel over batch, 2 batches per core on 8 cores.
"""

import sys

sys.path.insert(0, "/opt/trn_rl_repo")

import numpy as np
from contextlib import ExitStack

try:
    import jax

    jax.config.update("jax_compilation_cache_dir", "/var/tmp/jax-bass-cache")
    jax.config.update("jax_persistent_cache_min_compile_time_secs", 0.0)
    jax.config.update("jax_persistent_cache_min_entry_size_bytes", -1)
except Exception:
    pass

import concourse.bass as bass
import concourse.mybir as mybir
import concourse.tile as tile
from concourse.bass import ts, ds
from concourse.bass_utils import run_bass_kernel_spmd
from concourse.masks import make_identity

F32 = mybir.dt.float32
F32R = mybir.dt.float32r

N_CORES = 8
BPC = 2          # batches per core
S = 512          # sequence length
D = 512          # d_model
H = 8            # heads
DK = 64          # head dim
DC = 21          # dtw channels
NT = 4           # 128-row tiles per 512
Exp = mybir.ActivationFunctionType.Exp
Ident = mybir.ActivationFunctionType.Identity
Copy = mybir.ActivationFunctionType.Copy


def build_program():
    nc = bass.Bass("TRN2", debug=False, num_devices=N_CORES)

    q_d = nc.dram_tensor("q_in", [BPC, S, D], F32, kind="ExternalInput").ap()
    k_d = nc.dram_tensor("k_in", [BPC, S, D], F32, kind="ExternalInput").ap()
    v_d = nc.dram_tensor("v_in", [BPC, S, D], F32, kind="ExternalInput").ap()
    dtw_d = nc.dram_tensor("dtw", [BPC, DC, S, S], F32, kind="ExternalInput").ap()
    wq_d = nc.dram_tensor("wqT", [D, D], F32, kind="ExternalInput").ap()
    wk_d = nc.dram_tensor("wkT", [D, D], F32, kind="ExternalInput").ap()
    wv_d = nc.dram_tensor("wvT", [D, D], F32, kind="ExternalInput").ap()
    wo_d = nc.dram_tensor("woT", [D, D], F32, kind="ExternalInput").ap()
    wdp_d = nc.dram_tensor("wdpack", [84, NT, 128], F32, kind="ExternalInput").ap()
    bq_d = nc.dram_tensor("bq8", [64, H], F32, kind="ExternalInput").ap()
    bk_d = nc.dram_tensor("bk128", [64, H], F32, kind="ExternalInput").ap()
    bv_d = nc.dram_tensor("bv128", [128, NT], F32, kind="ExternalInput").ap()
    bo_d = nc.dram_tensor("bo128", [128, NT], F32, kind="ExternalInput").ap()

    out_d = nc.dram_tensor("out", [BPC, S, D], F32, kind="ExternalOutput").ap()
    attn_d = nc.dram_tensor("attn", [BPC, H, S, S], F32, kind="ExternalOutput").ap()

    with tile.TileContext(nc) as tc, ExitStack() as ctx:
        const = ctx.enter_context(tc.tile_pool(name="const", bufs=1))
        raw_p = ctx.enter_context(tc.tile_pool(name="raw", bufs=2))
        xt_p = ctx.enter_context(tc.tile_pool(name="xt", bufs=2))
        proj_p = ctx.enter_context(tc.tile_pool(name="proj", bufs=2))
        dtw_p = ctx.enter_context(tc.tile_pool(name="dtwp", bufs=2))
        wst_p = ctx.enter_context(tc.tile_pool(name="wst", bufs=2))
        bstg_p = ctx.enter_context(tc.tile_pool(name="bstg", bufs=2))
        attn_p = ctx.enter_context(tc.tile_pool(name="attnp", bufs=2))
        at_p = ctx.enter_context(tc.tile_pool(name="atp", bufs=2))
        cct_p = ctx.enter_context(tc.tile_pool(name="cct", bufs=1))
        outt_p = ctx.enter_context(tc.tile_pool(name="outt", bufs=1))
        outs_p = ctx.enter_context(tc.tile_pool(name="outs", bufs=1))
        sum_p = ctx.enter_context(tc.tile_pool(name="sump", bufs=4))

        ps_big = ctx.enter_context(tc.tile_pool(name="psb", bufs=4, space="PSUM"))
        ps_pv = ctx.enter_context(tc.tile_pool(name="pspv", bufs=2, space="PSUM"))

        # ---- constants -------------------------------------------------
        ident_f = const.tile([128, 128], F32)
        make_identity(nc, ident_f[:])
        ident_r = const.tile([128, 128], F32R)
        nc.scalar.copy(ident_r[:], ident_f[:])

        wdp_raw = const.tile([84, NT, 128], F32)
        nc.sync.dma_start(wdp_raw[:], wdp_d[:])
        wdp = const.tile([84, NT, 128], F32R)
        nc.scalar.copy(wdp[:], wdp_raw[:])

        biases = const.tile([128, 2 * NT], F32)
        nc.sync.dma_start(biases[:, 0:NT], bv_d[:])
        nc.sync.dma_start(biases[:, NT : 2 * NT], bo_d[:])
        bv1 = biases[:, 0:NT]
        bo1 = biases[:, NT : 2 * NT]
        biases_qk = const.tile([64, 2 * H], F32)
        nc.sync.dma_start(biases_qk[:, 0:H], bq_d[:])
        nc.sync.dma_start(biases_qk[:, H : 2 * H], bk_d[:])
        bq8 = biases_qk[:, 0:H]
        bk1 = biases_qk[:, H : 2 * H]

        # weights, rounded to f32r via ACT copy
        weights = {}
        for name, wd in (("wq", wq_d), ("wk", wk_d), ("wv", wv_d), ("wo", wo_d)):
            wr = raw_p.tile([128, NT, D], F32, tag="xraw")
            nc.sync.dma_start(wr[:], wd.rearrange("(c p) o -> p c o", p=128))
            wt = const.tile([128, NT, D], F32R, tag=f"w_{name}")
            nc.scalar.copy(wt[:], wr[:])
            weights[name] = wt

        # ---------------------------------------------------------------
        def transpose_in(x_d, bi, with_cb=True):
            """Load x[bi] [S,D] and PE-transpose to [D-part, S-free] f32r tiles."""
            xr = raw_p.tile([128, NT, D], F32, tag="xraw")
            nc.sync.dma_start(xr[:], x_d[bi].rearrange("(tc p) i -> p tc i", p=128))
            xt = xt_p.tile([128, NT, S], F32R, tag="xt")
            for ic in range(NT):
                pt = ps_big.tile([128, S], F32, tag="psb")
                for tcj in range(NT):
                    nc.tensor.transpose(
                        pt[:, ts(tcj, 128)], xr[:, tcj, ds(ic * 128, 128)], ident_f[:]
                    )
                nc.scalar.copy(xt[:, ic, :], pt[:])
            return xt

        def project_T(xt, w, bias_ap, scale):
            """out.T = W @ x.T -> per-head [64, 8(h), tok] f32r."""
            yt = proj_p.tile([64, H, S], F32R, tag="projT")
            for h in range(H):
                pp = ps_big.tile([64, S], F32, tag="psh", bufs=2)
                for ic in range(NT):
                    nc.tensor.matmul(
                        pp[:],
                        w[:, ic, ds(64 * h, 64)],
                        xt[:, ic, :],
                        start=(ic == 0),
                        stop=(ic == NT - 1),
                    )
                nc.scalar.activation(
                    out=yt[:, h, :], in_=pp[:], func=Ident,
                    bias=bias_ap[:, h : h + 1], scale=scale,
                )
            return yt

        def project_v(xt, w):
            """v = V @ Wv.T -> [tok-part(4), d'] f32r (bias folded in later via PV)."""
            yv = proj_p.tile([128, NT, D], F32R, tag="projV", bufs=1)
            for tc_ in range(NT):
                pp = ps_big.tile([128, D], F32, tag="psb")
                for ic in range(NT):
                    nc.tensor.matmul(
                        pp[:],
                        xt[:, ic, ds(tc_ * 128, 128)],
                        w[:, ic, :],
                        start=(ic == 0),
                        stop=(ic == NT - 1),
                    )
                nc.scalar.copy(yv[:, tc_, :], pp[:])
            return yv

        def emit_round(bi, t, r, wstage):
            """One dtw bias round: q-window [16*(8t+r), +16), all heads, all k."""
            q0 = 16 * (8 * t + r)
            dt_t = dtw_p.tile([84, NT, S], F32, tag="dtwt")
            for g in range(4):
                nc.sync.dma_start(
                    dt_t[:, g, :], dtw_d[bi, :, q0 + 4 * g : q0 + 4 * g + 4, :]
                )
            dt_r = dtw_p.tile([84, NT, S], F32R, tag="dtwr", bufs=1)
            nc.vector.tensor_copy(dt_r[:], dt_t[:])
            pr = ps_big.tile([128, S], F32, tag="psb")
            for g in range(4):
                nc.tensor.matmul(
                    pr[:],
                    wdp[:, g, :],
                    dt_r[:, g, :],
                    start=(g == 0),
                    stop=(g == 3),
                )
            nc.vector.tensor_copy(wstage[:, r, :], pr[:])

        # ---------------------------------------------------------------
        for bi in range(BPC):
            QT = transpose_in(q_d, bi)
            qT = project_T(QT, weights["wq"], bq8, 0.125)
            KT = transpose_in(k_d, bi)
            kT = project_T(KT, weights["wk"], bk1, 1.0)
            VT = transpose_in(v_d, bi)
            vv = project_v(VT, weights["wv"])

            wstages = {}
            # prologue: window 0 rounds
            wstages[0] = wst_p.tile([128, 8, S], F32R, tag="wstage", name="wstage0")
            for r in range(8):
                emit_round(bi, 0, r, wstages[0])

            concatT = cct_p.tile([128, NT, NT, 128], F32R, tag="cct")

            for t in range(NT):
                if t + 1 < NT:
                    wstages[t + 1] = wst_p.tile([128, 8, S], F32R, tag="wstage", name=f"wstage{t+1}")
                pv_even = ps_pv.tile([64, NT * 128], F32, tag="pve", bufs=1)
                pv_odd = ps_pv.tile([64, NT * 128], F32, tag="pvo", bufs=1)
                for h in range(H):
                    # bias scatter for (h, t): one DMA from wstage
                    bstg = bstg_p.tile([128, S], F32R, tag="bstg")
                    for a_ in range(8):
                        nc.sync.dma_start(
                            bstg[16 * a_ : 16 * a_ + 16, :],
                            wstages[t][h::8, a_, :],
                        )
                    # scores + bias add
                    ps_sc = ps_big.tile([128, S], F32, tag="psb")
                    nc.tensor.matmul(
                        ps_sc[:],
                        qT[:, h, ts(t, 128)],
                        kT[:, h, :],
                        start=True,
                        stop=False,
                    )
                    nc.tensor.matmul(
                        ps_sc[:], ident_r[:], bstg[:], start=False, stop=True,
                        skip_group_check=True,
                    )
                    # softmax (no max-subtract; scores are O(10))
                    a_un = attn_p.tile([128, S], F32, tag="a_un")
                    sums = sum_p.tile([128, 2], F32, tag="sums")
                    nc.scalar.activation(
                        out=a_un[:], in_=ps_sc[:], func=Exp,
                        scale=1.0, accum_out=sums[:, 0:1],
                    )
                    nc.vector.reciprocal(sums[:, 1:2], sums[:, 0:1])
                    nc.vector.tensor_scalar_mul(a_un[:], a_un[:], sums[:, 1:2])
                    nc.sync.dma_start(attn_d[bi, h, ts(t, 128), :], a_un[:])
                    # transpose attn tile -> [k, q]
                    ps_at = ps_big.tile([128, S], F32, tag="psb")
                    for kc in range(NT):
                        nc.tensor.transpose(
                            ps_at[:, ts(kc, 128)], a_un[:, ts(kc, 128)], ident_f[:]
                        )
                    aT = at_p.tile([128, NT, 128], F32R, tag="aT")
                    nc.scalar.copy(aT[:], ps_at[:])
                    # PV: outT_h[d, q-slice] += v[kc,:].T-contract
                    pv_dst = pv_even if h % 2 == 0 else pv_odd
                    for kc in range(NT):
                        nc.tensor.matmul(
                            pv_dst[:, ds((h // 2) * 128, 128)],
                            vv[:, kc, ds(64 * h, 64)],
                            aT[:, kc, :],
                            start=(kc == 0),
                            stop=(kc == NT - 1),
                            skip_group_check=True,
                        )
                    # interleave next window's dtw rounds
                    if t + 1 < NT:
                        emit_round(bi, t + 1, h, wstages[t + 1])
                # copy PV psum -> concatT with bv bias, per head-pair
                for pair in range(NT):
                    nc.scalar.activation(
                        out=concatT[0:64, pair, t, :], in_=pv_even[:, ts(pair, 128)],
                        func=Ident, bias=bv1[0:64, pair : pair + 1], scale=1.0,
                    )
                    nc.scalar.activation(
                        out=concatT[64:128, pair, t, :], in_=pv_odd[:, ts(pair, 128)],
                        func=Ident, bias=bv1[64:128, pair : pair + 1], scale=1.0,
                    )

            # output projection: out.T = Wo @ concat.T
            outT = outt_p.tile([128, NT, S], F32, tag="outT")
            for oc in range(NT):
                po = ps_big.tile([128, S], F32, tag="psb")
                for ic in range(NT):
                    nc.tensor.matmul(
                        po[:],
                        weights["wo"][:, ic, ds(oc * 128, 128)],
                        concatT[:, ic, :, :].rearrange("p a b -> p (a b)"),
                        start=(ic == 0),
                        stop=(ic == NT - 1),
                    )
                nc.scalar.activation(
                    out=outT[:, oc, :], in_=po[:], func=Ident,
                    bias=bo1[:, oc : oc + 1], scale=1.0,
                )
            # transpose back to [q, o] and store
            for qc in range(NT):
                pq = ps_big.tile([128, S], F32, tag="psb")
                for oc in range(NT):
                    nc.tensor.transpose(
                        pq[:, ts(oc, 128)], outT[:, oc, ts(qc, 128)], ident_f[:]
                    )
                ob = outs_p.tile([128, S], F32, tag="outsb")
                nc.scalar.copy(ob[:], pq[:])
                nc.sync.dma_start(out_d[bi, ts(qc, 128), :], ob[:])

    _split_limited_waits(nc)
    return nc


def _split_limited_waits(nc):
    """walrus codegen allows only one sync-wait on Matmult/DMACopy
    instructions; move extra waits onto preceding same-engine NoOps."""
    k = 0
    limited = ("InstMatmult", "InstMatmultMx", "InstDMACopy", "InstDmaTransposeAnt")
    for fn in nc.m.functions:
        for bb in fn.blocks:
            il = bb.instructions
            new = []
            for inst in il:
                si = inst.sync_info
                if (
                    type(inst).__name__ != "InstNoOp"
                    and si is not None
                    and len(si.on_wait) > 1
                ):
                    waits = list(si.on_wait)
                    for w in waits[:-1]:
                        nop = mybir.InstNoOp(name=f"I-waitfix-{k}", ins=[], outs=[])
                        k += 1
                        nop.engine = inst.engine
                        nop.sync_info = mybir.SyncInfo(on_wait=[w], on_update=[])
                        new.append(nop)
                    inst.sync_info = mybir.SyncInfo(
                        on_wait=waits[-1:], on_update=list(si.on_update)
                    )
                new.append(inst)
            il.clear()
            il.extend(new)
    return k


_NC = None


def _get_program():
    global _NC
    if _NC is None:
        _NC = build_program()
    return _NC


def kernel(Q, K, V, G, dtw_cost, Wq, bq, Wk, bk, Wv, bv, Wd, bd, Wo, bo):
    Q = np.ascontiguousarray(Q, np.float32)
    K = np.ascontiguousarray(K, np.float32)
    V = np.ascontiguousarray(V, np.float32)
    dtw_cost = np.ascontiguousarray(dtw_cost, np.float32)

    wqT = np.ascontiguousarray(np.asarray(Wq, np.float32).T)
    wkT = np.ascontiguousarray(np.asarray(Wk, np.float32).T)
    wvT = np.ascontiguousarray(np.asarray(Wv, np.float32).T)
    woT = np.ascontiguousarray(np.asarray(Wo, np.float32).T)

    Wd = np.asarray(Wd, np.float32)
    wdpack = np.zeros((84, NT, 128), np.float32)
    for c in range(DC):
        for j in range(4):
            for g in range(4):
                wdpack[c * 4 + j, g, 32 * g + 8 * j : 32 * g + 8 * j + 8] = Wd[:, c]

    bq8 = np.ascontiguousarray((np.asarray(bq, np.float32) / 8.0).reshape(H, 64).T)
    bk1 = np.ascontiguousarray(np.asarray(bk, np.float32).reshape(H, 64).T)
    bv1 = np.ascontiguousarray(np.asarray(bv, np.float32).reshape(NT, 128).T)
    bo1 = np.ascontiguousarray(np.asarray(bo, np.float32).reshape(NT, 128).T)
    # bd[h] is constant over k for fixed (h, q), so it cancels in softmax
    # (softmax(x + c) == softmax(x)); it is dropped entirely.

    bs = Q.shape[0]
    in_maps = []
    for core in range(N_CORES):
        b0 = core * BPC
        in_maps.append(
            {
                "q_in": Q[b0 : b0 + BPC],
                "k_in": K[b0 : b0 + BPC],
                "v_in": V[b0 : b0 + BPC],
                "dtw": dtw_cost[b0 : b0 + BPC],
                "wqT": wqT, "wkT": wkT, "wvT": wvT, "woT": woT,
                "wdpack": wdpack,
                "bq8": bq8, "bk128": bk1, "bv128": bv1, "bo128": bo1,
            }
        )

    nc = _get_program()
    res = run_bass_kernel_spmd(nc, in_maps, list(range(N_CORES)))
    global LAST_RESULTS
    LAST_RESULTS = res

    out = np.empty((bs, S, D), np.float32)
    attn = np.empty((bs, H, S, S), np.float32)
    for core in range(N_CORES):
        b0 = core * BPC
        out[b0 : b0 + BPC] = res.results[core]["out"]
        attn[b0 : b0 + BPC] = res.results[core]["attn"]
    return out, attn
